# revision 1
# baseline (speedup 1.0000x reference)
"""Single-head causal attention, distributed across 8 TRN2 NeuronCores.

Reference computation (fp32):
    Q = x @ Wq.T; K = x @ Wk.T; V = x @ Wv.T        # x [B=4, T=4096, C=768], W* [H=64, C]
    out = softmax(causal(Q @ K.T / sqrt(C))) @ V     # out [B, T, H]

Sharding: 8 cores = 4 batches x 2 query-halves. Core c handles batch b=c//2,
query rows [p*2048, (p+1)*2048) with p=c%2. Each core receives xT [768, 4096]
(pre-transposed on host): columns [0:2048) = "context" rows (batch rows
[0:2048), zeros for p=0 since it has no context), columns [2048:4096) = the
core's own 2048 rows. SPMD-uniform program; the only per-core variation is
data: a gate bias (0.0 visible / -50.0 masked) folded into the exp() so p=0
cores numerically kill their context block (exp(s-50)*2048 ~ 4e-19).

Kernel layout: scores are computed transposed, St[k_par, q_free] =
matmul(lhsT=Kt[:,kb], rhs=Qt), so softmax' P tiles feed the P@V matmul
directly as the moving operand with V|ones [k_par, 65] stationary; the
appended ones column accumulates the softmax denominator in the same PSUM
accumulation. Causal structure inside the own block is identical on all
cores => static per-diagonal masks; k-tiles fully above the diagonal are
statically skipped. All matmul I/O is float32r (fp32 bits, ~tf32 precision,
full-rate PE streaming).
"""

import numpy as np

B, T, C, H = 4, 4096, 768, 64
TQ = 2048          # own query rows per core
NKC = 8            # 512-wide column chunks of xT
NQC = 4            # 512-wide query chunks
NKB = 32           # 128-wide k tiles
CTXB = 16          # k tiles in the context region
SCALE = float(C) ** -0.5

DEFAULT_CFG = dict(
    pad_st=False,    # pad St contract to 128 (zeros in Kt rows 64:128)
    stp_bufs=3,      # PSUM buffers for score tiles
    pexp_bufs=1,     # SBUF buffers per staged P tile tag
    ptr_sep=True,    # small transpose PSUM tiles in their own tag
    stp_cols=512,    # score-tile width (exp instruction granularity)
    structure="split",  # "split" | "inter"
    pv_stop_each=False,  # close the PV accumulation group after every matmul
    pv_banks=1,      # number of PV PSUM accumulators (summed at finalize)
    exp_x=1,         # emit exp N times (marginal-cost probe)
    st_x=1,          # emit each St matmul N times (marginal-cost probe)
)


def build_bass(niter: int = 1, ablate: frozenset = frozenset(), **cfg_over):
    import concourse.bacc as bacc
    import concourse.mybir as mybir
    from concourse import tile
    from concourse.masks import make_identity

    cfg = dict(DEFAULT_CFG)
    cfg.update(cfg_over)
    pad = cfg["pad_st"]
    stp_cols = cfg["stp_cols"]
    nhalf = stp_cols // 512
    ptr_tag = "ptr" if cfg["ptr_sep"] else "pvq"

    fp32 = mybir.dt.float32
    f32r = mybir.dt.float32r
    Exp = mybir.ActivationFunctionType.Exp

    nc = bacc.Bacc("TRN2", target_bir_lowering=False, num_devices=8)
    xT_d = nc.dram_tensor("xT", [C, T], fp32, kind="ExternalInput")
    wT_d = nc.dram_tensor("wT", [C, 4 * H], fp32, kind="ExternalInput")
    gate_d = nc.dram_tensor("gate", [128, 1], fp32, kind="ExternalInput")
    out_d = nc.dram_tensor("out", [TQ, H], fp32, kind="ExternalOutput")

    with tile.TileContext(nc) as tc:
        with (
            tc.tile_pool(name="const", bufs=1) as constp,
            tc.tile_pool(name="data", bufs=1) as datap,
            tc.tile_pool(name="work", bufs=3) as workp,
            tc.tile_pool(name="ps", bufs=2, space="PSUM") as psp,
        ):
            def body(_iv=None):
                w_sb = constp.tile([128, 6, 4 * H], f32r, tag="w")
                nc.sync.dma_start(
                    w_sb[:], wT_d.ap().rearrange("(a p) n -> p a n", p=128).bitcast(f32r)
                )
                gate_sb = constp.tile([128, 1], fp32, tag="gate")
                nc.sync.dma_start(gate_sb[:], gate_d.ap())
                id_sb = constp.tile([128, 128], fp32, tag="id")
                make_identity(nc, id_sb[:])

                # 4 diagonal masks: dmask[d][pi, fj] = 1.0 iff fj - 128*d - pi >= 0
                dmasks = []
                for d in range(4):
                    dm = constp.tile([128, 512], fp32, tag=f"dmask{d}")
                    nc.gpsimd.memset(dm[:], 1.0)
                    nc.gpsimd.affine_select(
                        out=dm[:], in_=dm[:],
                        compare_op=mybir.AluOpType.is_ge,
                        fill=0.0, base=-128 * d,
                        channel_multiplier=-1, pattern=[[1, 512]],
                    )
                    dmasks.append(dm)

                kt_sb = datap.tile([128, T], f32r, tag="kt")
                if pad:
                    nc.vector.memset(kt_sb[64:128, :].bitcast(fp32), 0.0)
                qt_sb = datap.tile([128, TQ], f32r, tag="qt")
                vones = datap.tile([128, NKB, H + 1], f32r, tag="vones")
                nc.vector.memset(vones[:, :, H : H + 1].bitcast(fp32), 1.0)

                xts = []
                xtp_ctx = tc.tile_pool(name="xtp", bufs=1)
                xtp = xtp_ctx.__enter__()
                for kc in range(NKC):
                    xt = xtp.tile([128, 6, 512], f32r, tag=f"xt{kc}")
                    if "dma" not in ablate:
                        nc.sync.dma_start(
                            xt[:],
                            xT_d.ap()
                            .rearrange("(a p) n -> p a n", p=128)[
                                :, :, 512 * kc : 512 * (kc + 1)
                            ].bitcast(f32r),
                        )
                    if "dma" in ablate:
                        nc.vector.memset(xt[:, :, 0:1].bitcast(fp32), 0.5)
                    xts.append(xt)

                # K^T and V projections, 512 columns at a time.
                for kc in range(0 if "proj" not in ablate else NKC, NKC):
                    pskv = psp.tile([128, 512], fp32, tag="stkv",
                                    bufs=cfg["stp_bufs"])
                    for ct in range(6):
                        nc.tensor.matmul(
                            pskv[:],
                            lhsT=w_sb[:, ct, 2 * H : 4 * H],
                            rhs=xts[kc][:, ct, :],
                            start=(ct == 0), stop=(ct == 5),
                        )
                    nc.vector.tensor_copy(
                        kt_sb[0:64, 512 * kc : 512 * (kc + 1)], pskv[0:64, :]
                    )
                    vt_sb = workp.tile([64, 512], fp32, tag="vt")
                    nc.vector.tensor_copy(vt_sb[:], pskv[64:128, :])
                    for j in range(4):
                        kb = 4 * kc + j
                        pst = psp.tile([128, H], fp32, tag=ptr_tag, bufs=2)
                        nc.tensor.transpose(
                            pst[:], vt_sb[:, 128 * j : 128 * (j + 1)], id_sb[0:64, 0:64]
                        )
                        nc.vector.tensor_copy(vones[:, kb, 0:H], pst[:])

                # Q^T projection (Wq pre-scaled by 1/sqrt(C) on host;
                # packed twice so rows 64:127 duplicate rows 0:63).
                for qc in range(0 if "proj" not in ablate else NQC, NQC):
                    psq = psp.tile([128, 512], fp32, tag="pvq", bufs=2)
                    for ct in range(6):
                        nc.tensor.matmul(
                            psq[:],
                            lhsT=w_sb[:, ct, 0 : 2 * H],
                            rhs=xts[4 + qc][:, ct, :],
                            start=(ct == 0), stop=(ct == 5),
                        )
                    nc.scalar.copy(qt_sb[:, 512 * qc : 512 * (qc + 1)], psq[:])

                xtp_ctx.__exit__(None, None, None)
                pep_ctx = tc.tile_pool(name="pep", bufs=1)
                pep = pep_ctx.__enter__()

                if "proj" in ablate:
                    nc.vector.memset(kt_sb[:].bitcast(fp32), 0.5)
                    nc.vector.memset(qt_sb[:].bitcast(fp32), 0.5)
                    nc.vector.memset(vones[:, :, 0:H].bitcast(fp32), 0.5)

                krows = slice(0, 128) if pad else slice(0, 64)

                # Attention: St = Kt_blk.T @ Qt_chunk -> exp -> mask -> PV.
                nbank = cfg["pv_banks"]
                inter = cfg["structure"] == "inter"
                for qc in range(NQC):
                    n_kb = CTXB + 4 * qc + 4
                    ngrp = n_kb // nhalf
                    pvs_banks = [
                        psp.tile([H + 1, 512], fp32, tag="pvq", bufs=2,
                                 name=f"pv{i}")
                        for i in range(nbank)
                    ]
                    if "pv" in ablate:
                        for pvb in pvs_banks:
                            nc.vector.memset(pvb[:], 1.0)
                    pexps = []
                    started = [False] * nbank

                    def pv_mm(pr, half, pexp):
                        if "pv" in ablate:
                            return
                        kb = nhalf * pr + half
                        i = (nhalf * pr + half) % nbank
                        last = pr == ngrp - 1 and half == nhalf - 1
                        nc.tensor.matmul(
                            pvs_banks[i][:],
                            lhsT=vones[:, kb, :],
                            rhs=pexp[:, 512 * half : 512 * (half + 1)],
                            start=not started[i],
                            stop=(cfg["pv_stop_each"] or last
                                  or (inter and nbank > 1)),
                            skip_group_check=True,
                        )
                        started[i] = True

                    for pr in range(ngrp):
                        stp = psp.tile([128, stp_cols], fp32, tag="stkv",
                                       bufs=cfg["stp_bufs"])
                        pexp = pep.tile([128, stp_cols], f32r,
                                        tag=f"pexp{pr if not inter else pr % 4}",
                                        bufs=cfg["pexp_bufs"])
                        if "st" in ablate:
                            nc.vector.memset(stp[:, 0:1], 0.1)
                        for half in range(nhalf if "st" not in ablate else 0):
                            kb = nhalf * pr + half
                            for _x in range(cfg["st_x"]):
                                nc.tensor.matmul(
                                    stp[:, 512 * half : 512 * (half + 1)],
                                    lhsT=kt_sb[krows, 128 * kb : 128 * (kb + 1)],
                                    rhs=qt_sb[krows, 512 * qc : 512 * (qc + 1)],
                                    start=True, stop=True, skip_group_check=True,
                                )
                        if "exp" in ablate:
                            nc.vector.memset(pexp[:].bitcast(fp32), 0.5)
                        if "exp" not in ablate:
                            for _x in range(cfg["exp_x"]):
                                nc.scalar.activation(
                                    pexp[:], stp[:], Exp,
                                    bias=(gate_sb[:] if (nhalf * pr + nhalf - 1) < CTXB
                                          else 0.0),
                                )
                        for half in range(nhalf):
                            kb = nhalf * pr + half
                            b = kb - CTXB
                            if 4 * qc <= b <= 4 * qc + 3 and "mask" not in ablate:
                                nc.vector.tensor_mul(
                                    pexp[:, 512 * half : 512 * (half + 1)],
                                    pexp[:, 512 * half : 512 * (half + 1)],
                                    dmasks[b - 4 * qc][:],
                                )
                            if inter:
                                pv_mm(pr, half, pexp)
                        pexps.append(pexp)
                    if not inter:
                        for pr in range(ngrp):
                            for half in range(nhalf):
                                pv_mm(pr, half, pexps[pr])
                    pv = pvs_banks[0]
                    for pvb in pvs_banks[1:]:
                        nc.vector.tensor_add(pv[:], pv[:], pvb[:])
                    # normalize + transpose back to [q, h]
                    pvs = workp.tile([H + 1, 512], fp32, tag="pvs")
                    nc.vector.tensor_copy(pvs[:], pv[:])
                    for j in range(4):
                        pst2 = psp.tile([128, H + 1], fp32, tag=ptr_tag, bufs=2)
                        nc.tensor.transpose(
                            pst2[:],
                            pvs[:, 128 * j : 128 * (j + 1)],
                            id_sb[0 : H + 1, 0 : H + 1],
                        )
                        rec = workp.tile([128, 1], fp32, tag="rec")
                        nc.vector.reciprocal(rec[:], pst2[:, H : H + 1])
                        outt = workp.tile([128, H], fp32, tag="outt")
                        nc.vector.tensor_scalar_mul(outt[:], pst2[:, 0:H], rec[:])
                        r0 = 512 * qc + 128 * j
                        nc.sync.dma_start(out_d.ap()[r0 : r0 + 128, :], outt[:])
                pep_ctx.__exit__(None, None, None)

            if niter == 1:
                body()
            else:
                with tc.For_i(0, niter) as iv:
                    body(iv)

    nc.compile()
    return nc


_NC_CACHE = {}


def _get_nc(niter: int = 1):
    if niter not in _NC_CACHE:
        _NC_CACHE[niter] = build_bass(niter)
    return _NC_CACHE[niter]


def make_in_maps(x, Wq, Wk, Wv):
    wqs = Wq.T * SCALE
    wT = np.concatenate([wqs, wqs, Wk.T, Wv.T], axis=1).astype(np.float32)
    wT = np.ascontiguousarray(wT)
    in_maps = []
    for c in range(8):
        b, p = c // 2, c % 2
        xT = np.zeros((C, T), np.float32)
        if p == 1:
            xT[:, 0:TQ] = x[b, 0:TQ, :].T
        xT[:, TQ:T] = x[b, p * TQ : (p + 1) * TQ, :].T
        gate = np.full((128, 1), 0.0 if p == 1 else -50.0, np.float32)
        in_maps.append(
            {"xT": np.ascontiguousarray(xT), "wT": wT, "gate": gate}
        )
    return in_maps


def kernel(x, Wq, Wk, Wv):
    from concourse.bass_utils import run_bass_kernel_spmd

    x = np.asarray(x, np.float32)
    nc = _get_nc(1)
    in_maps = make_in_maps(x, np.asarray(Wq), np.asarray(Wk), np.asarray(Wv))
    res = run_bass_kernel_spmd(nc, in_maps, core_ids=list(range(8)), trace=False)
    out = np.empty((B, T, H), np.float32)
    for c in range(8):
        b, p = c // 2, c % 2
        out[b, p * TQ : (p + 1) * TQ, :] = res.results[c]["out"]
    return out



# revision 4
# speedup vs baseline: 2.3540x; 2.3540x over previous
"""Single-head causal attention, distributed across 8 TRN2 NeuronCores.

Reference computation (fp32):
    Q = x @ Wq.T; K = x @ Wk.T; V = x @ Wv.T        # x [B=4, T=4096, C=768], W* [H=64, C]
    out = softmax(causal(Q @ K.T / sqrt(C))) @ V     # out [B, T, H]

Sharding: 8 cores = 4 batches x 2 query-halves. Core c handles batch b=c//2,
query rows [p*2048, (p+1)*2048) with p=c%2. Each core receives xT [768, 4096]
(pre-transposed on host): columns [0:2048) = "context" rows (batch rows
[0:2048), zeros for p=0 since it has no context), columns [2048:4096) = the
core's own 2048 rows. SPMD-uniform program; the only per-core variation is
data: a gate bias (0.0 visible / -50.0 masked) folded into the exp() so p=0
cores numerically kill their context block (exp(s-50)*2048 ~ 4e-19).

Kernel layout: scores are computed transposed, St[k_par, q_free] =
matmul(lhsT=Kt[:,kb], rhs=Qt), so softmax' P tiles feed the P@V matmul
directly as the moving operand with V|ones [k_par, 65] stationary; the
appended ones column accumulates the softmax denominator in the same PSUM
accumulation. Causal structure inside the own block is identical on all
cores => static per-diagonal masks; k-tiles fully above the diagonal are
statically skipped. All matmul I/O is float32r (fp32 bits, ~tf32 precision,
full-rate PE streaming).
"""

import numpy as np

B, T, C, H = 4, 4096, 768, 64
TQ = 2048          # own query rows per core
NKC = 8            # 512-wide column chunks of xT
NQC = 4            # 512-wide query chunks
NKB = 32           # 128-wide k tiles
CTXB = 16          # k tiles in the context region
SCALE = float(C) ** -0.5

DEFAULT_CFG = dict(
    pad_st=False,    # pad St contract to 128 (zeros in Kt rows 64:128)
    stp_bufs=3,      # PSUM buffers for score tiles
    pexp_bufs=1,     # SBUF buffers per staged P tile tag
    ptr_sep=True,    # small transpose PSUM tiles in their own tag
    stp_cols=512,    # score-tile width (exp instruction granularity)
    structure="split",  # "split" | "inter"
    pv_stop_each=False,  # close the PV accumulation group after every matmul
    pv_banks=1,      # number of PV PSUM accumulators (summed at finalize)
    exp_x=1,         # emit exp N times (marginal-cost probe)
    st_x=1,          # emit each St matmul N times (marginal-cost probe)
    pv_x=1,          # emit each PV matmul N times (numerics-neutral: N/D scales)
    dma_x=1,         # issue each xT DMA N times (marginal-cost probe)
)


def build_bass(niter: int = 1, ablate: frozenset = frozenset(), **cfg_over):
    import concourse.bacc as bacc
    import concourse.mybir as mybir
    from concourse import tile
    from concourse.masks import make_identity

    cfg = dict(DEFAULT_CFG)
    cfg.update(cfg_over)
    pad = cfg["pad_st"]
    stp_cols = cfg["stp_cols"]
    nhalf = stp_cols // 512
    ptr_tag = "ptr" if cfg["ptr_sep"] else "pvq"

    fp32 = mybir.dt.float32
    f32r = mybir.dt.float32r
    Exp = mybir.ActivationFunctionType.Exp

    nc = bacc.Bacc("TRN2", target_bir_lowering=False, num_devices=8)
    xT_d = nc.dram_tensor("xT", [C, T], fp32, kind="ExternalInput")
    wT_d = nc.dram_tensor("wT", [C, 4 * H], fp32, kind="ExternalInput")
    gate_d = nc.dram_tensor("gate", [128, 1], fp32, kind="ExternalInput")
    out_d = nc.dram_tensor("out", [TQ, H], fp32, kind="ExternalOutput")

    with tile.TileContext(nc) as tc:
        with (
            tc.tile_pool(name="const", bufs=1) as constp,
            tc.tile_pool(name="data", bufs=1) as datap,
            tc.tile_pool(name="work", bufs=3) as workp,
            tc.tile_pool(name="ps", bufs=2, space="PSUM") as psp,
        ):
            def body(_iv=None):
                w_sb = constp.tile([128, 6, 4 * H], f32r, tag="w")
                nc.sync.dma_start(
                    w_sb[:], wT_d.ap().rearrange("(a p) n -> p a n", p=128).bitcast(f32r)
                )
                gate_sb = constp.tile([128, 1], fp32, tag="gate")
                nc.sync.dma_start(gate_sb[:], gate_d.ap())
                id_sb = constp.tile([128, 128], fp32, tag="id")
                make_identity(nc, id_sb[:])

                # 4 diagonal masks: dmask[d][pi, fj] = 1.0 iff fj - 128*d - pi >= 0
                dmasks = []
                for d in range(4):
                    dm = constp.tile([128, 512], fp32, tag=f"dmask{d}")
                    nc.gpsimd.memset(dm[:], 1.0)
                    nc.gpsimd.affine_select(
                        out=dm[:], in_=dm[:],
                        compare_op=mybir.AluOpType.is_ge,
                        fill=0.0, base=-128 * d,
                        channel_multiplier=-1, pattern=[[1, 512]],
                    )
                    dmasks.append(dm)

                kt_sb = datap.tile([128, T], f32r, tag="kt")
                if pad:
                    nc.vector.memset(kt_sb[64:128, :].bitcast(fp32), 0.0)
                qt_sb = datap.tile([128, TQ], f32r, tag="qt")
                vones = datap.tile([128, NKB, H + 1], f32r, tag="vones")
                nc.vector.memset(vones[:, :, H : H + 1].bitcast(fp32), 1.0)

                xts = []
                xtp_ctx = tc.tile_pool(name="xtp", bufs=1)
                xtp = xtp_ctx.__enter__()
                for kc in range(NKC):
                    xt = xtp.tile([128, 6, 512], f32r, tag=f"xt{kc}")
                    if "dma" not in ablate:
                        for _x in range(cfg["dma_x"]):
                            nc.sync.dma_start(
                                xt[:],
                                xT_d.ap()
                                .rearrange("(a p) n -> p a n", p=128)[
                                    :, :, 512 * kc : 512 * (kc + 1)
                                ].bitcast(f32r),
                            )
                    if "dma" in ablate:
                        nc.vector.memset(xt[:, :, 0:1].bitcast(fp32), 0.5)
                    xts.append(xt)

                # K^T and V projections, 512 columns at a time.
                for kc in range(0 if "proj" not in ablate else NKC, NKC):
                    pskv = psp.tile([128, 512], fp32, tag="stkv",
                                    bufs=cfg["stp_bufs"])
                    for ct in range(6):
                        nc.tensor.matmul(
                            pskv[:],
                            lhsT=w_sb[:, ct, 2 * H : 4 * H],
                            rhs=xts[kc][:, ct, :],
                            start=(ct == 0), stop=(ct == 5),
                        )
                    nc.vector.tensor_copy(
                        kt_sb[0:64, 512 * kc : 512 * (kc + 1)], pskv[0:64, :]
                    )
                    vt_sb = workp.tile([64, 512], fp32, tag="vt")
                    nc.vector.tensor_copy(vt_sb[:], pskv[64:128, :])
                    for j in range(4):
                        kb = 4 * kc + j
                        pst = psp.tile([128, H], fp32, tag=ptr_tag, bufs=2)
                        nc.tensor.transpose(
                            pst[:], vt_sb[:, 128 * j : 128 * (j + 1)], id_sb[0:64, 0:64]
                        )
                        nc.vector.tensor_copy(vones[:, kb, 0:H], pst[:])

                # Q^T projection (Wq pre-scaled by 1/sqrt(C) on host;
                # packed twice so rows 64:127 duplicate rows 0:63).
                for qc in range(0 if "proj" not in ablate else NQC, NQC):
                    psq = psp.tile([128, 512], fp32, tag="pvq", bufs=2)
                    for ct in range(6):
                        nc.tensor.matmul(
                            psq[:],
                            lhsT=w_sb[:, ct, 0 : 2 * H],
                            rhs=xts[4 + qc][:, ct, :],
                            start=(ct == 0), stop=(ct == 5),
                        )
                    nc.scalar.copy(qt_sb[:, 512 * qc : 512 * (qc + 1)], psq[:])

                xtp_ctx.__exit__(None, None, None)
                pep_ctx = tc.tile_pool(name="pep", bufs=1)
                pep = pep_ctx.__enter__()

                if "proj" in ablate:
                    nc.vector.memset(kt_sb[:].bitcast(fp32), 0.5)
                    nc.vector.memset(qt_sb[:].bitcast(fp32), 0.5)
                    nc.vector.memset(vones[:, :, 0:H].bitcast(fp32), 0.5)

                krows = slice(0, 128) if pad else slice(0, 64)

                # Attention: St = Kt_blk.T @ Qt_chunk -> exp -> mask -> PV.
                nbank = cfg["pv_banks"]
                inter = cfg["structure"] == "inter"
                for qc in range(NQC):
                    n_kb = CTXB + 4 * qc + 4
                    ngrp = n_kb // nhalf
                    pvs_banks = [
                        psp.tile([H + 1, 512], fp32, tag="pvq", bufs=2,
                                 name=f"pv{i}")
                        for i in range(nbank)
                    ]
                    if "pv" in ablate:
                        for pvb in pvs_banks:
                            nc.vector.memset(pvb[:], 1.0)
                    pexps = []
                    started = [False] * nbank

                    def pv_mm(pr, half, pexp):
                        if "pv" in ablate:
                            return
                        kb = nhalf * pr + half
                        i = (nhalf * pr + half) % nbank
                        last = pr == ngrp - 1 and half == nhalf - 1
                        for _x in range(cfg["pv_x"]):
                            x_last = _x == cfg["pv_x"] - 1
                            nc.tensor.matmul(
                                pvs_banks[i][:],
                                lhsT=vones[:, kb, :],
                                rhs=pexp[:, 512 * half : 512 * (half + 1)],
                                start=not started[i],
                                stop=x_last and (cfg["pv_stop_each"] or last
                                                 or (inter and nbank > 1)),
                                skip_group_check=True,
                            )
                            started[i] = True

                    for pr in range(ngrp):
                        stp = psp.tile([128, stp_cols], fp32, tag="stkv",
                                       bufs=cfg["stp_bufs"])
                        pexp = pep.tile([128, stp_cols], f32r,
                                        tag=f"pexp{pr if not inter else pr % 4}",
                                        bufs=cfg["pexp_bufs"])
                        if "st" in ablate:
                            nc.vector.memset(stp[:, 0:1], 0.1)
                        for half in range(nhalf if "st" not in ablate else 0):
                            kb = nhalf * pr + half
                            for _x in range(cfg["st_x"]):
                                nc.tensor.matmul(
                                    stp[:, 512 * half : 512 * (half + 1)],
                                    lhsT=kt_sb[krows, 128 * kb : 128 * (kb + 1)],
                                    rhs=qt_sb[krows, 512 * qc : 512 * (qc + 1)],
                                    start=True, stop=True, skip_group_check=True,
                                )
                        if "exp" in ablate:
                            nc.vector.memset(pexp[:].bitcast(fp32), 0.5)
                        if "exp" not in ablate:
                            for _x in range(cfg["exp_x"]):
                                nc.scalar.activation(
                                    pexp[:], stp[:], Exp,
                                    bias=(gate_sb[:] if (nhalf * pr + nhalf - 1) < CTXB
                                          else 0.0),
                                )
                        for half in range(nhalf):
                            kb = nhalf * pr + half
                            b = kb - CTXB
                            if 4 * qc <= b <= 4 * qc + 3 and "mask" not in ablate:
                                nc.vector.tensor_mul(
                                    pexp[:, 512 * half : 512 * (half + 1)],
                                    pexp[:, 512 * half : 512 * (half + 1)],
                                    dmasks[b - 4 * qc][:],
                                )
                            if inter:
                                pv_mm(pr, half, pexp)
                        pexps.append(pexp)
                    if not inter:
                        for pr in range(ngrp):
                            for half in range(nhalf):
                                pv_mm(pr, half, pexps[pr])
                    pv = pvs_banks[0]
                    for pvb in pvs_banks[1:]:
                        nc.vector.tensor_add(pv[:], pv[:], pvb[:])
                    # normalize + transpose back to [q, h]
                    pvs = workp.tile([H + 1, 512], fp32, tag="pvs")
                    nc.vector.tensor_copy(pvs[:], pv[:])
                    for j in range(4):
                        pst2 = psp.tile([128, H + 1], fp32, tag=ptr_tag, bufs=2)
                        nc.tensor.transpose(
                            pst2[:],
                            pvs[:, 128 * j : 128 * (j + 1)],
                            id_sb[0 : H + 1, 0 : H + 1],
                        )
                        rec = workp.tile([128, 1], fp32, tag="rec")
                        nc.vector.reciprocal(rec[:], pst2[:, H : H + 1])
                        outt = workp.tile([128, H], fp32, tag="outt")
                        nc.vector.tensor_scalar_mul(outt[:], pst2[:, 0:H], rec[:])
                        r0 = 512 * qc + 128 * j
                        nc.sync.dma_start(out_d.ap()[r0 : r0 + 128, :], outt[:])
                pep_ctx.__exit__(None, None, None)

            if niter == 1:
                body()
            else:
                with tc.For_i(0, niter) as iv:
                    body(iv)

    nc.compile()
    return nc


_NC_CACHE = {}


def _get_nc(niter: int = 1):
    if niter not in _NC_CACHE:
        _NC_CACHE[niter] = build_bass(niter)
    return _NC_CACHE[niter]


def make_in_maps(x, Wq, Wk, Wv):
    wqs = Wq.T * SCALE
    wT = np.concatenate([wqs, wqs, Wk.T, Wv.T], axis=1).astype(np.float32)
    wT = np.ascontiguousarray(wT)
    in_maps = []
    for c in range(8):
        b, p = c // 2, c % 2
        xT = np.zeros((C, T), np.float32)
        if p == 1:
            xT[:, 0:TQ] = x[b, 0:TQ, :].T
        xT[:, TQ:T] = x[b, p * TQ : (p + 1) * TQ, :].T
        gate = np.full((128, 1), 0.0 if p == 1 else -50.0, np.float32)
        in_maps.append(
            {"xT": np.ascontiguousarray(xT), "wT": wT, "gate": gate}
        )
    return in_maps


def kernel(x, Wq, Wk, Wv):
    from concourse.bass_utils import run_bass_kernel_spmd

    x = np.asarray(x, np.float32)
    nc = _get_nc(1)
    in_maps = make_in_maps(x, np.asarray(Wq), np.asarray(Wk), np.asarray(Wv))
    res = run_bass_kernel_spmd(nc, in_maps, core_ids=list(range(8)), trace=False)
    out = np.empty((B, T, H), np.float32)
    for c in range(8):
        b, p = c // 2, c % 2
        out[b, p * TQ : (p + 1) * TQ, :] = res.results[c]["out"]
    return out

